# revision 1
# baseline (speedup 1.0000x reference)
"""Attention-pooling kernel for TRN2 (8 NeuronCores, data-parallel over batch).

Computes, per batch b:
    scores = seeds @ x[b].T          # [M, S]
    weights = softmax(scores, -1)
    out[b] = weights @ x[b]          # [M, D]

Sharding: batch B=32 split 4-per-core across 8 cores; seeds replicated.

Per-core pipeline (all bf16 on-chip, f32 PSUM accumulation):
  - SWDGE cast-DMA loads x tiles HBM f32 -> SBUF bf16 (cast rides the DMA).
  - PE transposes x 128x128 blocks (bf16, FWL weight loads) -> psum -> DVE
    copies to SBUF as x^T chunks.
  - scores: 4 accumulating matmuls lhsT=seedsT chunk [128,16], rhs=xT [128,512].
  - exp on ACT straight out of PSUM, with fused accum_out row-sums
    (no max subtraction: scores = seeds.x are bounded ~|8|, exp is safe in f32).
  - PE transposes exp [16,128] -> expT [128,16]; pooled matmuls are 4-way
    column-tiled (concurrent PE col-groups), partials accumulate in
    psum[32q:32q+16, :] over the whole batch.
  - batch end: reduce partials + recip(sum) on DVE, scale, DMA out f32.
  - Stages are software-pipelined (C(i-2), B(i-1), A(i)) so PE never waits
    on the ACT/DVE round trip of the same macro-tile.
"""

from contextlib import ExitStack

import numpy as np

import concourse.mybir as mybir
import concourse.tile as tile
from concourse import bacc
from concourse.bass_utils import run_bass_kernel_spmd
from concourse.masks import make_identity

N_CORES = 8
B, S, D, M = 32, 8192, 512, 16
S_MACRO = 512          # s rows per macro-tile
T_SUB = S_MACRO // 128  # 128-row subtiles per macro-tile
DC = D // 128           # 128-col d chunks

f32 = mybir.dt.float32
bf16 = mybir.dt.bfloat16


def kernel_body(tc, out_ap, x_ap, seeds_ap, b_loc, s):
    nc = tc.nc
    n_mac = s // S_MACRO
    with ExitStack() as ctx:
        const = ctx.enter_context(tc.tile_pool(name="const", bufs=1))
        xp = ctx.enter_context(tc.tile_pool(name="xp", bufs=7))
        xtp = ctx.enter_context(tc.tile_pool(name="xtp", bufs=4))
        ep = ctx.enter_context(tc.tile_pool(name="ep", bufs=4))
        etp = ctx.enter_context(tc.tile_pool(name="etp", bufs=4))
        statp = ctx.enter_context(tc.tile_pool(name="statp", bufs=4))
        outp = ctx.enter_context(tc.tile_pool(name="outp", bufs=2))
        ps_xt = ctx.enter_context(tc.tile_pool(name="ps_xt", bufs=4, space="PSUM"))
        ps_sc = ctx.enter_context(tc.tile_pool(name="ps_sc", bufs=1, space="PSUM"))
        ps_et = ctx.enter_context(tc.tile_pool(name="ps_et", bufs=1, space="PSUM"))
        ps_pl = ctx.enter_context(tc.tile_pool(name="ps_pl", bufs=2, space="PSUM"))

        ident = const.tile([128, 128], bf16)
        make_identity(nc, ident)

        # seeds -> bf16 -> seedsT [d, m] chunks, [128, DC*M] (dc-major)
        seeds_bf = const.tile([M, D], bf16)
        nc.gpsimd.dma_start(out=seeds_bf[:], in_=seeds_ap)
        ps_st = ps_et.tile([128, DC * M], bf16, tag="et", name="et")
        for dc in range(DC):
            nc.tensor.transpose(
                ps_st[:, dc * M:(dc + 1) * M],
                seeds_bf[:, dc * 128:(dc + 1) * 128],
                ident[:M, :M],
            )
        seedsT = const.tile([128, DC * M], bf16)
        nc.vector.tensor_copy(seedsT[:], ps_st[:])

        # x view: [b, n, p, q, d] with s = n*S_MACRO + p*T_SUB + q.
        # Partition p holds T_SUB consecutive s rows -> 8KB contiguous HBM
        # reads per partition (4x fewer DMA packets). The s-order inside a
        # macro is a fixed permutation; softmax is permutation-invariant and
        # scores/exp/pooled all use the same block mapping, so it cancels.
        x_r = x_ap.rearrange("b (n p q) d -> b n p q d", p=128, q=T_SUB)

        # Software-pipelined across all (batch, macro) pairs:
        #   stage A(i): DMA load, PE x-transposes, DVE psum->sbuf copies
        #   stage B(i): scores matmuls, ACT exp (+row-sum)
        #   stage C(i): PE exp-transposes, DVE copy, pooled matmuls, finalize
        # C runs 2 macros behind A so PE never waits on the ACT/DVE round
        # trip of the same macro.
        macros = [(bb, n) for bb in range(b_loc) for n in range(n_mac)]
        NM = len(macros)
        st = {}  # per-macro live tiles
        batch = {}  # per-batch state: sums tile, pool psum

        def stage_a(i):
            bb, n = macros[i]
            x_bf = xp.tile([128, T_SUB, D], bf16, tag="x", name="x_bf")
            nc.gpsimd.dma_start(out=x_bf[:], in_=x_r[bb, n])
            xt_sb = xtp.tile([128, DC, S_MACRO], bf16, tag="xt", name="xt")
            for ph in range(DC // 2):  # 2 dc chunks per psum bank
                xt_ps = ps_xt.tile([128, 2 * S_MACRO], bf16, tag="xt", name="xt")
                for dch in range(2):
                    dc = ph * 2 + dch
                    for t in range(T_SUB):
                        nc.tensor.transpose(
                            xt_ps[:, dch * S_MACRO + t * 128:
                                  dch * S_MACRO + (t + 1) * 128],
                            x_bf[:, t, dc * 128:(dc + 1) * 128],
                            ident[:],
                        )
                nc.vector.tensor_copy(xt_sb[:, ph * 2:(ph + 1) * 2, :], xt_ps[:])
            st[i] = {"x": x_bf, "xt": xt_sb}

        def stage_b(i):
            bb, n = macros[i]
            if n == 0:
                batch[bb] = {"sums": statp.tile([M, n_mac], f32, tag="sums", name="sums")}
            xt_sb = st[i]["xt"]
            sc_ps = ps_sc.tile([M, S_MACRO], f32, tag="sc", name="sc")
            for dc in range(DC):
                nc.tensor.matmul(
                    sc_ps[:],
                    lhsT=seedsT[:, dc * M:(dc + 1) * M],
                    rhs=xt_sb[:, dc, :],
                    start=(dc == 0),
                    stop=(dc == DC - 1),
                )
            e_bf = ep.tile([M, S_MACRO], bf16, tag="e", name="e_bf")
            nc.scalar.activation(
                e_bf[:], sc_ps[:], mybir.ActivationFunctionType.Exp,
                accum_out=batch[bb]["sums"][:, n:n + 1],
            )
            st[i]["e"] = e_bf
            if n == n_mac - 1:
                # sums complete after this exp; compute recip early so the
                # batch-end finalize chain starts without it
                total = statp.tile([M, 1], f32, tag="tot", name="tot")
                nc.vector.reduce_sum(
                    total[:], batch[bb]["sums"][:], axis=mybir.AxisListType.X)
                recip = statp.tile([M, 1], f32, tag="rec", name="rec")
                nc.vector.reciprocal(recip[:], total[:])
                batch[bb]["recip"] = recip

        def stage_c1(i):
            # expT transposes + small DVE copy; runs while scores(i+1) stream
            bb, n = macros[i]
            if n == 0:
                batch[bb]["pl"] = ps_pl.tile([128, D], f32, tag="pl", name="pl")
            e_bf = st[i]["e"]
            et_ps = ps_et.tile([128, T_SUB * M], bf16, tag="et", name="et")
            for t in range(T_SUB):
                nc.tensor.transpose(
                    et_ps[:, t * M:(t + 1) * M],
                    e_bf[:, t * 128:(t + 1) * 128],
                    ident[:M, :M],
                )
            et_sb = etp.tile([128, T_SUB * M], bf16, tag="et", name="et")
            nc.vector.tensor_copy(et_sb[:], et_ps[:])
            st[i]["et"] = et_sb

        def stage_c2(i):
            bb, n = macros[i]
            x_bf = st[i]["x"]
            et_sb = st[i]["et"]
            pool_ps = batch[bb]["pl"]
            # 4-way column-tiled: each q-block runs in its own 32-col group
            # of the PE array with its own XBUS stream; the 4 streams run
            # concurrently. Partial sums land on psum partitions 32q..32q+15
            # and are reduced once per batch.
            for t in range(T_SUB):
                nc.tensor.matmul(
                    pool_ps[32 * t:32 * t + M, :],
                    lhsT=et_sb[:, t * M:(t + 1) * M],
                    rhs=x_bf[:, t, :],
                    start=(n == 0),
                    stop=(n == n_mac - 1),
                    tile_position=(0, 32 * t),
                    skip_group_check=True,
                )
            del st[i]
            if n == n_mac - 1:
                recip = batch[bb]["recip"]
                # fused: o = sum_q partial_q * recip, one op per partial
                o_sb = outp.tile([M, D], f32, tag="o", name="o_sb")
                nc.vector.tensor_scalar_mul(o_sb[:], pool_ps[0:M, :], recip[:])
                for q in range(1, T_SUB):
                    nc.vector.scalar_tensor_tensor(
                        o_sb[:], pool_ps[32 * q:32 * q + M, :], recip[:],
                        o_sb[:], op0=mybir.AluOpType.mult,
                        op1=mybir.AluOpType.add,
                    )
                nc.scalar.dma_start(out=out_ap[bb], in_=o_sb[:])
                del batch[bb]

        for i in range(NM + 2):
            if 2 <= i <= NM + 1:
                stage_c1(i - 2)
            if 1 <= i <= NM:
                stage_b(i - 1)
            if 2 <= i <= NM + 1:
                stage_c2(i - 2)
            if i < NM:
                stage_a(i)


def build_bass(b_loc, s):
    nc = bacc.Bacc(
        "TRN2", target_bir_lowering=False, debug=False, num_devices=N_CORES
    )
    x_d = nc.dram_tensor("x", [b_loc, s, D], f32, kind="ExternalInput")
    seeds_d = nc.dram_tensor("seeds", [M, D], f32, kind="ExternalInput")
    out_d = nc.dram_tensor("out", [b_loc, M, D], f32, kind="ExternalOutput")
    with tile.TileContext(nc) as tc:
        kernel_body(tc, out_d.ap(), x_d.ap(), seeds_d.ap(), b_loc, s)
    nc.compile()
    return nc


_cached = {}


def get_nc(b_loc, s):
    key = (b_loc, s)
    if key not in _cached:
        _cached[key] = build_bass(b_loc, s)
    return _cached[key]


def kernel(x, seeds, trace=False):
    assert x.shape == (B, S, D) and seeds.shape == (M, D)
    x = np.asarray(x, dtype=np.float32)
    seeds = np.asarray(seeds, dtype=np.float32)
    b_loc = B // N_CORES
    nc = get_nc(b_loc, S)
    in_maps = [
        {
            "x": np.ascontiguousarray(x[i * b_loc:(i + 1) * b_loc]),
            "seeds": seeds,
        }
        for i in range(N_CORES)
    ]
    res = run_bass_kernel_spmd(
        nc, in_maps, core_ids=list(range(N_CORES)), trace=trace
    )
    out = np.concatenate([r["out"] for r in res.results], axis=0)
    if trace:
        kernel.last_result = res
    return out.astype(np.float32)


kernel.last_result = None



# revision 6
# speedup vs baseline: 64168.0898x; 64168.0898x over previous
"""Attention-pooling kernel for TRN2 (8 NeuronCores, data-parallel over batch).

Computes, per batch b:
    scores = seeds @ x[b].T          # [M, S]
    weights = softmax(scores, -1)
    out[b] = weights @ x[b]          # [M, D]

Sharding: batch B=32 split 4-per-core across 8 cores; seeds replicated.

Per-core pipeline (all bf16 on-chip, f32 PSUM accumulation):
  - SWDGE cast-DMA loads x tiles HBM f32 -> SBUF bf16 (cast rides the DMA).
  - PE transposes x 128x128 blocks (bf16, FWL weight loads) -> psum -> DVE
    copies to SBUF as x^T chunks.
  - scores: 4 accumulating matmuls lhsT=seedsT chunk [128,16], rhs=xT [128,512].
  - exp on ACT straight out of PSUM, with fused accum_out row-sums
    (no max subtraction: scores = seeds.x are bounded ~|8|, exp is safe in f32).
  - PE transposes exp [16,128] -> expT [128,16]; pooled matmuls are 4-way
    column-tiled (concurrent PE col-groups), partials accumulate in
    psum[32q:32q+16, :] over the whole batch.
  - batch end: reduce partials + recip(sum) on DVE, scale, DMA out f32.
  - Stages are software-pipelined (C(i-2), B(i-1), A(i)) so PE never waits
    on the ACT/DVE round trip of the same macro-tile.
"""

from contextlib import ExitStack

import numpy as np

import concourse.mybir as mybir
import concourse.tile as tile
from concourse import bacc
from concourse.bass_utils import run_bass_kernel_spmd
from concourse.masks import make_identity

N_CORES = 8
B, S, D, M = 32, 8192, 512, 16
S_MACRO = 512          # s rows per macro-tile
T_SUB = S_MACRO // 128  # 128-row subtiles per macro-tile
DC = D // 128           # 128-col d chunks
Q_CHUNK = 16            # s rows per partition per DMA chunk (4 MB HBM read)
MAC_PER_CHUNK = Q_CHUNK // T_SUB
XP_BUFS = 4             # chunk tiles in flight (16 KB/partition each)

f32 = mybir.dt.float32
bf16 = mybir.dt.bfloat16


def kernel_body(tc, out_ap, x_ap, seeds_ap, b_loc, s):
    nc = tc.nc
    n_mac = s // S_MACRO
    with ExitStack() as ctx:
        const = ctx.enter_context(tc.tile_pool(name="const", bufs=1))
        xp = ctx.enter_context(tc.tile_pool(name="xp", bufs=XP_BUFS))
        xtp = ctx.enter_context(tc.tile_pool(name="xtp", bufs=4))
        ep = ctx.enter_context(tc.tile_pool(name="ep", bufs=4))
        etp = ctx.enter_context(tc.tile_pool(name="etp", bufs=4))
        statp = ctx.enter_context(tc.tile_pool(name="statp", bufs=4))
        outp = ctx.enter_context(tc.tile_pool(name="outp", bufs=2))
        ps_xt = ctx.enter_context(tc.tile_pool(name="ps_xt", bufs=4, space="PSUM"))
        ps_sc = ctx.enter_context(tc.tile_pool(name="ps_sc", bufs=1, space="PSUM"))
        ps_et = ctx.enter_context(tc.tile_pool(name="ps_et", bufs=1, space="PSUM"))
        ps_pl = ctx.enter_context(tc.tile_pool(name="ps_pl", bufs=2, space="PSUM"))

        ident = const.tile([128, 128], bf16)
        make_identity(nc, ident)

        # seeds -> bf16 -> seedsT [d, m] chunks, [128, DC*M] (dc-major)
        seeds_bf = const.tile([M, D], bf16)
        nc.gpsimd.dma_start(out=seeds_bf[:], in_=seeds_ap)
        ps_st = ps_et.tile([128, DC * M], bf16, tag="et", name="et")
        for dc in range(DC):
            nc.tensor.transpose(
                ps_st[:, dc * M:(dc + 1) * M],
                seeds_bf[:, dc * 128:(dc + 1) * 128],
                ident[:M, :M],
            )
        seedsT = const.tile([128, DC * M], bf16)
        nc.vector.tensor_copy(seedsT[:], ps_st[:])

        # x view: [b, n, p, q, d] with s = n*(128*Q_CHUNK) + p*Q_CHUNK + q.
        # One DMA chunk = 2048 s rows = 4 MB of HBM reads, 32 KB contiguous
        # per partition (vs 1 MB/8 KB at macro granularity: 4x fewer SWDGE
        # emissions, and >=4 MB transfers run near the 358 GB/s HBM ceiling
        # instead of ~341). The s-order is a fixed permutation; softmax is
        # permutation-invariant and scores/exp/pooled all use the same block
        # mapping, so it cancels.
        x_r = x_ap.rearrange("b (n p q) d -> b n p q d", p=128, q=Q_CHUNK)
        n_chunks = b_loc * (s // (128 * Q_CHUNK))

        # Software-pipelined across all (batch, macro) pairs:
        #   DMA: one 4 MB cast-load per chunk (4 macros), PREFETCH ahead
        #   stage A(i): PE x-transposes, DVE psum->sbuf copies
        #   stage B(i): scores matmuls, ACT exp (+row-sum)
        #   stage C(i): PE exp-transposes, DVE copy, pooled matmuls, finalize
        # C runs 2 macros behind A so PE never waits on the ACT/DVE round
        # trip of the same macro.
        macros = [(bb, n) for bb in range(b_loc) for n in range(n_mac)]
        NM = len(macros)
        st = {}  # per-macro live tiles
        batch = {}  # per-batch state: sums tile, pool psum
        chunk_tiles = {}  # chunk idx -> x_bf tile

        def stage_dma(c):
            bb, nn_ = divmod(c, s // (128 * Q_CHUNK))
            x_bf = xp.tile([128, Q_CHUNK, D], bf16, tag="x", name="x_bf")
            nc.gpsimd.dma_start(out=x_bf[:], in_=x_r[bb, nn_])
            chunk_tiles[c] = x_bf

        def stage_a(i):
            qb = (i % MAC_PER_CHUNK) * T_SUB
            x_bf = chunk_tiles[i // MAC_PER_CHUNK]
            xt_sb = xtp.tile([128, DC, S_MACRO], bf16, tag="xt", name="xt")
            for ph in range(DC // 2):  # 2 dc chunks per psum bank
                xt_ps = ps_xt.tile([128, 2 * S_MACRO], bf16, tag="xt", name="xt")
                for dch in range(2):
                    dc = ph * 2 + dch
                    for t in range(T_SUB):
                        nc.tensor.transpose(
                            xt_ps[:, dch * S_MACRO + t * 128:
                                  dch * S_MACRO + (t + 1) * 128],
                            x_bf[:, qb + t, dc * 128:(dc + 1) * 128],
                            ident[:],
                        )
                nc.vector.tensor_copy(xt_sb[:, ph * 2:(ph + 1) * 2, :], xt_ps[:])
            st[i] = {"x": x_bf, "xt": xt_sb, "qb": qb}

        def stage_b(i):
            bb, n = macros[i]
            if n == 0:
                batch[bb] = {"sums": statp.tile([M, n_mac], f32, tag="sums", name="sums")}
            xt_sb = st[i]["xt"]
            sc_ps = ps_sc.tile([M, S_MACRO], f32, tag="sc", name="sc")
            for dc in range(DC):
                nc.tensor.matmul(
                    sc_ps[:],
                    lhsT=seedsT[:, dc * M:(dc + 1) * M],
                    rhs=xt_sb[:, dc, :],
                    start=(dc == 0),
                    stop=(dc == DC - 1),
                )
            e_bf = ep.tile([M, S_MACRO], bf16, tag="e", name="e_bf")
            nc.scalar.activation(
                e_bf[:], sc_ps[:], mybir.ActivationFunctionType.Exp,
                accum_out=batch[bb]["sums"][:, n:n + 1],
            )
            st[i]["e"] = e_bf
            if n == n_mac - 1:
                # sums complete after this exp; compute recip early so the
                # batch-end finalize chain starts without it
                total = statp.tile([M, 1], f32, tag="tot", name="tot")
                nc.vector.reduce_sum(
                    total[:], batch[bb]["sums"][:], axis=mybir.AxisListType.X)
                recip = statp.tile([M, 1], f32, tag="rec", name="rec")
                nc.vector.reciprocal(recip[:], total[:])
                batch[bb]["recip"] = recip

        def stage_c1(i):
            # expT transposes + small DVE copy; runs while scores(i+1) stream
            bb, n = macros[i]
            if n == 0:
                batch[bb]["pl"] = ps_pl.tile([128, D], f32, tag="pl", name="pl")
            e_bf = st[i]["e"]
            et_ps = ps_et.tile([128, T_SUB * M], bf16, tag="et", name="et")
            for t in range(T_SUB):
                nc.tensor.transpose(
                    et_ps[:, t * M:(t + 1) * M],
                    e_bf[:, t * 128:(t + 1) * 128],
                    ident[:M, :M],
                )
            et_sb = etp.tile([128, T_SUB * M], bf16, tag="et", name="et")
            nc.vector.tensor_copy(et_sb[:], et_ps[:])
            st[i]["et"] = et_sb

        def stage_c2(i):
            bb, n = macros[i]
            x_bf = st[i]["x"]
            qb = st[i]["qb"]
            et_sb = st[i]["et"]
            pool_ps = batch[bb]["pl"]
            # 4-way column-tiled: each q-block runs in its own 32-col group
            # of the PE array with its own XBUS stream; the 4 streams run
            # concurrently. Partial sums land on psum partitions 32q..32q+15
            # and are reduced once per batch.
            for t in range(T_SUB):
                nc.tensor.matmul(
                    pool_ps[32 * t:32 * t + M, :],
                    lhsT=et_sb[:, t * M:(t + 1) * M],
                    rhs=x_bf[:, qb + t, :],
                    start=(n == 0),
                    stop=(n == n_mac - 1),
                    tile_position=(0, 32 * t),
                    skip_group_check=True,
                )
            del st[i]
            if i % MAC_PER_CHUNK == MAC_PER_CHUNK - 1:
                del chunk_tiles[i // MAC_PER_CHUNK]
            if n == n_mac - 1:
                recip = batch[bb]["recip"]
                # fused: o = sum_q partial_q * recip, one op per partial
                o_sb = outp.tile([M, D], f32, tag="o", name="o_sb")
                nc.vector.tensor_scalar_mul(o_sb[:], pool_ps[0:M, :], recip[:])
                for q in range(1, T_SUB):
                    nc.vector.scalar_tensor_tensor(
                        o_sb[:], pool_ps[32 * q:32 * q + M, :], recip[:],
                        o_sb[:], op0=mybir.AluOpType.mult,
                        op1=mybir.AluOpType.add,
                    )
                nc.scalar.dma_start(out=out_ap[bb], in_=o_sb[:])
                del batch[bb]

        PREFETCH = XP_BUFS - 1  # chunks issued ahead of compute
        for c in range(min(PREFETCH, n_chunks)):
            stage_dma(c)
        for i in range(NM + 2):
            if i % MAC_PER_CHUNK == 0 and i // MAC_PER_CHUNK + PREFETCH < n_chunks:
                stage_dma(i // MAC_PER_CHUNK + PREFETCH)
            if 2 <= i <= NM + 1:
                stage_c1(i - 2)
            if 1 <= i <= NM:
                stage_b(i - 1)
            if 2 <= i <= NM + 1:
                stage_c2(i - 2)
            if i < NM:
                stage_a(i)


def build_bass(b_loc, s):
    nc = bacc.Bacc(
        "TRN2", target_bir_lowering=False, debug=False, num_devices=N_CORES
    )
    x_d = nc.dram_tensor("x", [b_loc, s, D], f32, kind="ExternalInput")
    seeds_d = nc.dram_tensor("seeds", [M, D], f32, kind="ExternalInput")
    out_d = nc.dram_tensor("out", [b_loc, M, D], f32, kind="ExternalOutput")
    with tile.TileContext(nc) as tc:
        kernel_body(tc, out_d.ap(), x_d.ap(), seeds_d.ap(), b_loc, s)
    nc.compile()
    return nc


_cached = {}


def get_nc(b_loc, s):
    key = (b_loc, s)
    if key not in _cached:
        _cached[key] = build_bass(b_loc, s)
    return _cached[key]


def kernel(x, seeds, trace=False):
    assert x.shape == (B, S, D) and seeds.shape == (M, D)
    x = np.asarray(x, dtype=np.float32)
    seeds = np.asarray(seeds, dtype=np.float32)
    b_loc = B // N_CORES
    nc = get_nc(b_loc, S)
    in_maps = [
        {
            "x": np.ascontiguousarray(x[i * b_loc:(i + 1) * b_loc]),
            "seeds": seeds,
        }
        for i in range(N_CORES)
    ]
    res = run_bass_kernel_spmd(
        nc, in_maps, core_ids=list(range(N_CORES)), trace=trace
    )
    out = np.concatenate([r["out"] for r in res.results], axis=0)
    if trace:
        kernel.last_result = res
    return out.astype(np.float32)


kernel.last_result = None



# revision 7
# speedup vs baseline: 65356.5803x; 1.0185x over previous
"""Attention-pooling kernel for TRN2 (8 NeuronCores, data-parallel over batch).

Computes, per batch b:
    scores = seeds @ x[b].T          # [M, S]
    weights = softmax(scores, -1)
    out[b] = weights @ x[b]          # [M, D]

Sharding: batch B=32 split 4-per-core across 8 cores; seeds replicated.

Per-core pipeline (all bf16 on-chip, f32 PSUM accumulation):
  - SWDGE cast-DMA loads x HBM f32 -> SBUF bf16 in tapered chunks
    (1-4 MB: small at the very start to fill the pipeline early and at
    the very end to shrink the post-stream compute tail; 4 MB in the
    middle where only line rate matters). The x stream is the critical
    path (~67 MB at ~337 GB/s effective HBM rate = ~199 us), so the
    chunk DMAs are the first instructions on the gpsimd queue and the
    seeds load rides the HWDGE (scalar) queue instead.
  - PE transposes x 128x128 blocks (bf16) -> psum -> DVE copies to SBUF
    as x^T chunks.
  - scores: 4 accumulating matmuls lhsT=seedsT chunk [128,16], rhs=xT.
  - exp on ACT straight out of PSUM, with fused accum_out row-sums
    (no max subtraction: scores = seeds.x are bounded ~|8|, exp is safe
    in f32).
  - PE transposes exp [16,128] -> expT [128,16]; pooled matmuls are
    4-way column-tiled (concurrent PE col-groups), partials accumulate
    in psum[32q:32q+16, :] over the whole batch.
  - batch end: reduce partials + recip(sum) on DVE, scale, DMA out f32.
  - Stages are software-pipelined (C(i-2), B(i-1), A(i)) so PE never
    waits on the ACT/DVE round trip of the same macro-tile.
"""

from contextlib import ExitStack

import numpy as np

import concourse.mybir as mybir
import concourse.tile as tile
from concourse import bacc
from concourse.bass_utils import run_bass_kernel_spmd
from concourse.masks import make_identity

N_CORES = 8
B, S, D, M = 32, 8192, 512, 16
S_MACRO = 512           # s rows per macro-tile
T_SUB = S_MACRO // 128  # 128-row subtiles per macro-tile
DC = D // 128            # 128-col d chunks
Q_MAX = 16               # max q rows/partition per chunk tile (4 macros)
XP_BUFS = 5              # chunk tiles in flight (16 KB/partition each)

f32 = mybir.dt.float32
bf16 = mybir.dt.bfloat16


def chunk_sizes(bb, b_loc, n_mac):
    """Macro counts per DMA chunk for one batch (sums to n_mac).

    Small chunks at the global start (pipeline fill) and global end
    (short post-stream tail); 4-macro (4 MB) chunks in the middle.
    """
    first, last = bb == 0, bb == b_loc - 1
    if first and last:
        sizes = [1, 1, 2, 4, 4, 2, 1, 1]
    elif first:
        sizes = [1, 1, 2, 4, 4, 4]
    elif last:
        sizes = [4, 4, 4, 2, 1, 1]
    else:
        sizes = [4, 4, 4, 4]
    assert sum(sizes) == n_mac
    return sizes


def kernel_body(tc, out_ap, x_ap, seeds_ap, b_loc, s):
    nc = tc.nc
    n_mac = s // S_MACRO
    with ExitStack() as ctx:
        const = ctx.enter_context(tc.tile_pool(name="const", bufs=1))
        xp = ctx.enter_context(tc.tile_pool(name="xp", bufs=XP_BUFS))
        xtp = ctx.enter_context(tc.tile_pool(name="xtp", bufs=4))
        ep = ctx.enter_context(tc.tile_pool(name="ep", bufs=4))
        etp = ctx.enter_context(tc.tile_pool(name="etp", bufs=4))
        statp = ctx.enter_context(tc.tile_pool(name="statp", bufs=4))
        outp = ctx.enter_context(tc.tile_pool(name="outp", bufs=2))
        ps_xt = ctx.enter_context(tc.tile_pool(name="ps_xt", bufs=4, space="PSUM"))
        ps_sc = ctx.enter_context(tc.tile_pool(name="ps_sc", bufs=1, space="PSUM"))
        ps_et = ctx.enter_context(tc.tile_pool(name="ps_et", bufs=1, space="PSUM"))
        ps_pl = ctx.enter_context(tc.tile_pool(name="ps_pl", bufs=2, space="PSUM"))

        # x view per batch: [b, p, q, d] with s = p*(s/128) + q. Partition p
        # holds s/128=64 consecutive s rows (128 KB contiguous HBM), so any
        # q-slice is a contiguous per-partition read. The s-order is a fixed
        # permutation; softmax is permutation-invariant and scores/exp/pooled
        # all use the same block mapping, so it cancels.
        x_rb = x_ap.rearrange("b (p q) d -> b p q d", p=128)

        # chunk schedule: (bb, first macro in batch, n macros)
        sched = []
        for bb in range(b_loc):
            m0 = 0
            for sz in chunk_sizes(bb, b_loc, n_mac):
                sched.append((bb, m0, sz))
                m0 += sz
        first_of_chunk = {}  # global macro idx -> chunk idx
        macro_chunk = {}     # global macro idx -> (chunk idx, sub idx)
        for ci, (bb, m0, nm) in enumerate(sched):
            first_of_chunk[bb * n_mac + m0] = ci
            for j in range(nm):
                macro_chunk[bb * n_mac + m0 + j] = (ci, j)

        chunk_tiles = {}

        def stage_dma(c):
            bb, m0, nm = sched[c]
            x_bf = xp.tile([128, Q_MAX, D], bf16, tag="x", name="x_bf")
            nc.gpsimd.dma_start(
                out=x_bf[:, :nm * T_SUB, :],
                in_=x_rb[bb, :, m0 * T_SUB:(m0 + nm) * T_SUB, :],
            )
            chunk_tiles[c] = x_bf

        # x chunk DMAs are the first gpsimd-queue work: prefetch before
        # identity/seeds setup so SDMA starts streaming immediately.
        PREFETCH = XP_BUFS - 1
        for c in range(min(PREFETCH, len(sched))):
            stage_dma(c)

        ident = const.tile([128, 128], bf16)
        make_identity(nc, ident)

        # seeds on the HWDGE (scalar) queue, f32; DVE casts to bf16.
        seeds_f = const.tile([M, D], f32)
        nc.scalar.dma_start(out=seeds_f[:], in_=seeds_ap)
        seeds_bf = const.tile([M, D], bf16)
        nc.vector.tensor_copy(seeds_bf[:], seeds_f[:])

        # seeds -> seedsT [d, m] chunks, [128, DC*M] (dc-major)
        ps_st = ps_et.tile([128, DC * M], bf16, tag="et", name="et")
        for dc in range(DC):
            nc.tensor.transpose(
                ps_st[:, dc * M:(dc + 1) * M],
                seeds_bf[:, dc * 128:(dc + 1) * 128],
                ident[:M, :M],
            )
        seedsT = const.tile([128, DC * M], bf16)
        nc.vector.tensor_copy(seedsT[:], ps_st[:])

        # Software-pipelined across all (batch, macro) pairs:
        #   DMA: one cast-load per chunk, PREFETCH chunks ahead
        #   stage A(i): PE x-transposes, DVE psum->sbuf copies
        #   stage B(i): scores matmuls, ACT exp (+row-sum)
        #   stage C(i): PE exp-transposes, DVE copy, pooled matmuls, finalize
        # C runs 2 macros behind A so PE never waits on the ACT/DVE round
        # trip of the same macro.
        macros = [(bb, n) for bb in range(b_loc) for n in range(n_mac)]
        NM = len(macros)
        st = {}  # per-macro live tiles
        batch = {}  # per-batch state: sums tile, pool psum

        def stage_a(i):
            ci, j = macro_chunk[i]
            qb = j * T_SUB
            x_bf = chunk_tiles[ci]
            xt_sb = xtp.tile([128, DC, S_MACRO], bf16, tag="xt", name="xt")
            for ph in range(DC // 2):  # 2 dc chunks per psum bank
                xt_ps = ps_xt.tile([128, 2 * S_MACRO], bf16, tag="xt", name="xt")
                for dch in range(2):
                    dc = ph * 2 + dch
                    for t in range(T_SUB):
                        nc.tensor.transpose(
                            xt_ps[:, dch * S_MACRO + t * 128:
                                  dch * S_MACRO + (t + 1) * 128],
                            x_bf[:, qb + t, dc * 128:(dc + 1) * 128],
                            ident[:],
                        )
                nc.vector.tensor_copy(xt_sb[:, ph * 2:(ph + 1) * 2, :], xt_ps[:])
            st[i] = {"x": x_bf, "xt": xt_sb, "qb": qb}

        def stage_b(i):
            bb, n = macros[i]
            if n == 0:
                batch[bb] = {"sums": statp.tile([M, n_mac], f32, tag="sums", name="sums")}
            xt_sb = st[i]["xt"]
            sc_ps = ps_sc.tile([M, S_MACRO], f32, tag="sc", name="sc")
            for dc in range(DC):
                nc.tensor.matmul(
                    sc_ps[:],
                    lhsT=seedsT[:, dc * M:(dc + 1) * M],
                    rhs=xt_sb[:, dc, :],
                    start=(dc == 0),
                    stop=(dc == DC - 1),
                )
            e_bf = ep.tile([M, S_MACRO], bf16, tag="e", name="e_bf")
            nc.scalar.activation(
                e_bf[:], sc_ps[:], mybir.ActivationFunctionType.Exp,
                accum_out=batch[bb]["sums"][:, n:n + 1],
            )
            st[i]["e"] = e_bf
            if n == n_mac - 1:
                # sums complete after this exp; compute recip early so the
                # batch-end finalize chain starts without it
                total = statp.tile([M, 1], f32, tag="tot", name="tot")
                nc.vector.reduce_sum(
                    total[:], batch[bb]["sums"][:], axis=mybir.AxisListType.X)
                recip = statp.tile([M, 1], f32, tag="rec", name="rec")
                nc.vector.reciprocal(recip[:], total[:])
                batch[bb]["recip"] = recip

        def stage_c1(i):
            # expT transposes + small DVE copy; runs while scores(i+1) stream
            bb, n = macros[i]
            if n == 0:
                batch[bb]["pl"] = ps_pl.tile([128, D], f32, tag="pl", name="pl")
            e_bf = st[i]["e"]
            et_ps = ps_et.tile([128, T_SUB * M], bf16, tag="et", name="et")
            for t in range(T_SUB):
                nc.tensor.transpose(
                    et_ps[:, t * M:(t + 1) * M],
                    e_bf[:, t * 128:(t + 1) * 128],
                    ident[:M, :M],
                )
            et_sb = etp.tile([128, T_SUB * M], bf16, tag="et", name="et")
            nc.vector.tensor_copy(et_sb[:], et_ps[:])
            st[i]["et"] = et_sb

        def stage_c2(i):
            bb, n = macros[i]
            x_bf = st[i]["x"]
            qb = st[i]["qb"]
            et_sb = st[i]["et"]
            pool_ps = batch[bb]["pl"]
            # 4-way column-tiled: each q-block runs in its own 32-col group
            # of the PE array with its own XBUS stream; the 4 streams run
            # concurrently. Partial sums land on psum partitions 32q..32q+15
            # and are reduced once per batch.
            for t in range(T_SUB):
                nc.tensor.matmul(
                    pool_ps[32 * t:32 * t + M, :],
                    lhsT=et_sb[:, t * M:(t + 1) * M],
                    rhs=x_bf[:, qb + t, :],
                    start=(n == 0),
                    stop=(n == n_mac - 1),
                    tile_position=(0, 32 * t),
                    skip_group_check=True,
                )
            del st[i]
            ci, j = macro_chunk[i]
            if j == sched[ci][2] - 1:
                del chunk_tiles[ci]
            if n == n_mac - 1:
                recip = batch[bb]["recip"]
                # fused: o = sum_q partial_q * recip, one op per partial
                o_sb = outp.tile([M, D], f32, tag="o", name="o_sb")
                nc.vector.tensor_scalar_mul(o_sb[:], pool_ps[0:M, :], recip[:])
                for q in range(1, T_SUB):
                    nc.vector.scalar_tensor_tensor(
                        o_sb[:], pool_ps[32 * q:32 * q + M, :], recip[:],
                        o_sb[:], op0=mybir.AluOpType.mult,
                        op1=mybir.AluOpType.add,
                    )
                nc.scalar.dma_start(out=out_ap[bb], in_=o_sb[:])
                del batch[bb]

        for i in range(NM + 2):
            if i < NM and i in first_of_chunk:
                c = first_of_chunk[i] + PREFETCH
                if c < len(sched):
                    stage_dma(c)
            if 2 <= i <= NM + 1:
                stage_c1(i - 2)
            if 1 <= i <= NM:
                stage_b(i - 1)
            if 2 <= i <= NM + 1:
                stage_c2(i - 2)
            if i < NM:
                stage_a(i)


def build_bass(b_loc, s):
    nc = bacc.Bacc(
        "TRN2", target_bir_lowering=False, debug=False, num_devices=N_CORES
    )
    x_d = nc.dram_tensor("x", [b_loc, s, D], f32, kind="ExternalInput")
    seeds_d = nc.dram_tensor("seeds", [M, D], f32, kind="ExternalInput")
    out_d = nc.dram_tensor("out", [b_loc, M, D], f32, kind="ExternalOutput")
    with tile.TileContext(nc) as tc:
        kernel_body(tc, out_d.ap(), x_d.ap(), seeds_d.ap(), b_loc, s)
    nc.compile()
    return nc


_cached = {}


def get_nc(b_loc, s):
    key = (b_loc, s)
    if key not in _cached:
        _cached[key] = build_bass(b_loc, s)
    return _cached[key]


def kernel(x, seeds, trace=False):
    assert x.shape == (B, S, D) and seeds.shape == (M, D)
    x = np.asarray(x, dtype=np.float32)
    seeds = np.asarray(seeds, dtype=np.float32)
    b_loc = B // N_CORES
    nc = get_nc(b_loc, S)
    in_maps = [
        {
            "x": np.ascontiguousarray(x[i * b_loc:(i + 1) * b_loc]),
            "seeds": seeds,
        }
        for i in range(N_CORES)
    ]
    res = run_bass_kernel_spmd(
        nc, in_maps, core_ids=list(range(N_CORES)), trace=trace
    )
    out = np.concatenate([r["out"] for r in res.results], axis=0)
    if trace:
        kernel.last_result = res
    return out.astype(np.float32)


kernel.last_result = None


# revision 11
# speedup vs baseline: 65711.5132x; 1.0054x over previous
"""Attention-pooling kernel for TRN2 (8 NeuronCores, data-parallel over batch).

Computes, per batch b:
    scores = seeds @ x[b].T          # [M, S]
    weights = softmax(scores, -1)
    out[b] = weights @ x[b]          # [M, D]

Sharding: batch B=32 split 4-per-core across 8 cores; seeds replicated.

Per-core pipeline (all bf16 on-chip, f32 PSUM accumulation):
  - SWDGE cast-DMA loads x HBM f32 -> SBUF bf16 in tapered chunks
    (1-4 MB: small at the very start to fill the pipeline early and at
    the very end to shrink the post-stream compute tail; 4 MB in the
    middle where only line rate matters). The x stream is the critical
    path (~67 MB at ~337 GB/s effective HBM rate = ~199 us), so the
    chunk DMAs are the first instructions on the gpsimd queue and the
    seeds load rides the HWDGE (scalar) queue instead.
  - PE transposes x 128x128 blocks (bf16) -> psum -> DVE copies to SBUF
    as x^T chunks.
  - scores: 4 accumulating matmuls lhsT=seedsT chunk [128,16], rhs=xT.
  - exp on ACT straight out of PSUM, with fused accum_out row-sums
    (no max subtraction: scores = seeds.x are bounded ~|8|, exp is safe
    in f32).
  - PE transposes exp [16,128] -> expT [128,16]; pooled matmuls are
    4-way column-tiled (concurrent PE col-groups), partials accumulate
    in psum[32q:32q+16, :] over the whole batch.
  - batch end: reduce partials + recip(sum) on DVE, scale, DMA out f32.
  - Stages are software-pipelined (C(i-2), B(i-1), A(i)) so PE never
    waits on the ACT/DVE round trip of the same macro-tile.
"""

from contextlib import ExitStack

import numpy as np

import concourse.mybir as mybir
import concourse.tile as tile
from concourse import bacc
from concourse.bass_utils import run_bass_kernel_spmd
from concourse.masks import make_identity

N_CORES = 8
B, S, D, M = 32, 8192, 512, 16
S_MACRO = 512           # s rows per macro-tile
T_SUB = S_MACRO // 128  # 128-row subtiles per macro-tile
DC = D // 128            # 128-col d chunks
Q_MAX = 16               # max q rows/partition per chunk tile (4 macros)
XP_BUFS = 5              # chunk tiles in flight (16 KB/partition each)

f32 = mybir.dt.float32
bf16 = mybir.dt.bfloat16


def chunk_sizes(bb, b_loc, n_mac):
    """Macro counts per DMA chunk for one batch (sums to n_mac).

    Small chunks at the global start (pipeline fill) and global end
    (short post-stream tail); 4-macro (4 MB) chunks in the middle.
    """
    first, last = bb == 0, bb == b_loc - 1
    if first and last:
        sizes = [1, 1, 2, 4, 4, 2, 1, 1]
    elif first:
        sizes = [1, 1, 2, 4, 4, 4]
    elif last:
        sizes = [4, 4, 4, 2, 1, 1]
    else:
        sizes = [4, 4, 4, 4]
    assert sum(sizes) == n_mac
    return sizes


def kernel_body(tc, out_ap, x_ap, seeds_ap, b_loc, s):
    nc = tc.nc
    n_mac = s // S_MACRO
    with ExitStack() as ctx:
        const = ctx.enter_context(tc.tile_pool(name="const", bufs=1))
        xp = ctx.enter_context(tc.tile_pool(name="xp", bufs=XP_BUFS))
        xtp = ctx.enter_context(tc.tile_pool(name="xtp", bufs=4))
        ep = ctx.enter_context(tc.tile_pool(name="ep", bufs=4))
        etp = ctx.enter_context(tc.tile_pool(name="etp", bufs=4))
        statp = ctx.enter_context(tc.tile_pool(name="statp", bufs=4))
        outp = ctx.enter_context(tc.tile_pool(name="outp", bufs=2))
        ps_xt = ctx.enter_context(tc.tile_pool(name="ps_xt", bufs=4, space="PSUM"))
        ps_sc = ctx.enter_context(tc.tile_pool(name="ps_sc", bufs=1, space="PSUM"))
        ps_et = ctx.enter_context(tc.tile_pool(name="ps_et", bufs=1, space="PSUM"))
        ps_pl = ctx.enter_context(tc.tile_pool(name="ps_pl", bufs=2, space="PSUM"))

        # x view per batch: [b, p, q, d] with s = p*(s/128) + q. Partition p
        # holds s/128=64 consecutive s rows (128 KB contiguous HBM), so any
        # q-slice is a contiguous per-partition read. The s-order is a fixed
        # permutation; softmax is permutation-invariant and scores/exp/pooled
        # all use the same block mapping, so it cancels.
        x_rb = x_ap.rearrange("b (p q) d -> b p q d", p=128)

        # chunk schedule: (bb, first macro in batch, n macros)
        sched = []
        for bb in range(b_loc):
            m0 = 0
            for sz in chunk_sizes(bb, b_loc, n_mac):
                sched.append((bb, m0, sz))
                m0 += sz
        first_of_chunk = {}  # global macro idx -> chunk idx
        macro_chunk = {}     # global macro idx -> (chunk idx, sub idx)
        for ci, (bb, m0, nm) in enumerate(sched):
            first_of_chunk[bb * n_mac + m0] = ci
            for j in range(nm):
                macro_chunk[bb * n_mac + m0 + j] = (ci, j)

        chunk_tiles = {}

        def stage_dma(c):
            bb, m0, nm = sched[c]
            x_bf = xp.tile([128, Q_MAX, D], bf16, tag="x", name="x_bf")
            if c == 0:
                # First chunk rides HWDGE (sync queue) as raw f32 + DVE cast:
                # HWDGE needs no SWDGE descriptor-ring init, so the first
                # bytes land ~4 us earlier and compute starts sooner.
                x_f = const.tile([128, nm * T_SUB, D], f32, name="x_f0")
                nc.sync.dma_start(
                    out=x_f[:], in_=x_rb[bb, :, m0 * T_SUB:(m0 + nm) * T_SUB, :])
                nc.vector.tensor_copy(x_bf[:, :nm * T_SUB, :], x_f[:])
            else:
                nc.gpsimd.dma_start(
                    out=x_bf[:, :nm * T_SUB, :],
                    in_=x_rb[bb, :, m0 * T_SUB:(m0 + nm) * T_SUB, :],
                )
            chunk_tiles[c] = x_bf

        # x chunk DMAs are the first gpsimd-queue work: prefetch before
        # identity/seeds setup so SDMA starts streaming immediately.
        PREFETCH = XP_BUFS - 1
        for c in range(min(PREFETCH, len(sched))):
            stage_dma(c)

        ident = const.tile([128, 128], bf16)
        make_identity(nc, ident)

        # seeds on the HWDGE (scalar) queue, f32; DVE casts to bf16.
        seeds_f = const.tile([M, D], f32)
        nc.scalar.dma_start(out=seeds_f[:], in_=seeds_ap)
        seeds_bf = const.tile([M, D], bf16)
        nc.vector.tensor_copy(seeds_bf[:], seeds_f[:])

        # seeds -> seedsT [d, m] chunks, [128, DC*M] (dc-major)
        ps_st = ps_et.tile([128, DC * M], bf16, tag="et", name="et")
        for dc in range(DC):
            nc.tensor.transpose(
                ps_st[:, dc * M:(dc + 1) * M],
                seeds_bf[:, dc * 128:(dc + 1) * 128],
                ident[:M, :M],
            )
        seedsT = const.tile([128, DC * M], bf16)
        nc.vector.tensor_copy(seedsT[:], ps_st[:])

        # Software-pipelined across all (batch, macro) pairs:
        #   DMA: one cast-load per chunk, PREFETCH chunks ahead
        #   stage A(i): PE x-transposes, DVE psum->sbuf copies
        #   stage B(i): scores matmuls, ACT exp (+row-sum)
        #   stage C(i): PE exp-transposes, DVE copy, pooled matmuls, finalize
        # B runs 2 macros behind A and C 3 behind, so every cross-engine
        # round trip (PE->DVE xt copy->scores; ACT exp->eT) has a full
        # macro of slack and the PE never stalls mid-macro.
        macros = [(bb, n) for bb in range(b_loc) for n in range(n_mac)]
        NM = len(macros)
        st = {}  # per-macro live tiles
        batch = {}  # per-batch state: sums tile, pool psum

        def stage_a(i):
            ci, j = macro_chunk[i]
            qb = j * T_SUB
            x_bf = chunk_tiles[ci]
            xt_sb = xtp.tile([128, DC, S_MACRO], bf16, tag="xt", name="xt")
            for ph in range(DC // 2):  # 2 dc chunks per psum bank
                xt_ps = ps_xt.tile([128, 2 * S_MACRO], bf16, tag="xt", name="xt")
                for dch in range(2):
                    dc = ph * 2 + dch
                    for t in range(T_SUB):
                        nc.tensor.transpose(
                            xt_ps[:, dch * S_MACRO + t * 128:
                                  dch * S_MACRO + (t + 1) * 128],
                            x_bf[:, qb + t, dc * 128:(dc + 1) * 128],
                            ident[:],
                        )
                nc.vector.tensor_copy(xt_sb[:, ph * 2:(ph + 1) * 2, :], xt_ps[:])
            st[i] = {"x": x_bf, "xt": xt_sb, "qb": qb}

        def stage_b(i):
            bb, n = macros[i]
            if n == 0:
                batch[bb] = {"sums": statp.tile([M, n_mac], f32, tag="sums", name="sums")}
            xt_sb = st[i]["xt"]
            sc_ps = ps_sc.tile([M, S_MACRO], f32, tag="sc", name="sc")
            for dc in range(DC):
                nc.tensor.matmul(
                    sc_ps[:],
                    lhsT=seedsT[:, dc * M:(dc + 1) * M],
                    rhs=xt_sb[:, dc, :],
                    start=(dc == 0),
                    stop=(dc == DC - 1),
                )
            e_bf = ep.tile([M, S_MACRO], bf16, tag="e", name="e_bf")
            nc.scalar.activation(
                e_bf[:], sc_ps[:], mybir.ActivationFunctionType.Exp,
                accum_out=batch[bb]["sums"][:, n:n + 1],
            )
            st[i]["e"] = e_bf
            if n == n_mac - 1:
                # sums complete after this exp; compute recip early so the
                # batch-end finalize chain starts without it
                total = statp.tile([M, 1], f32, tag="tot", name="tot")
                nc.vector.reduce_sum(
                    total[:], batch[bb]["sums"][:], axis=mybir.AxisListType.X)
                recip = statp.tile([M, 1], f32, tag="rec", name="rec")
                nc.vector.reciprocal(recip[:], total[:])
                batch[bb]["recip"] = recip

        def stage_c1(i):
            # expT transposes + small DVE copy; runs while scores(i+1) stream
            bb, n = macros[i]
            if n == 0:
                batch[bb]["pl"] = ps_pl.tile([128, D], f32, tag="pl", name="pl")
            e_bf = st[i]["e"]
            et_ps = ps_et.tile([128, T_SUB * M], bf16, tag="et", name="et")
            for t in range(T_SUB):
                nc.tensor.transpose(
                    et_ps[:, t * M:(t + 1) * M],
                    e_bf[:, t * 128:(t + 1) * 128],
                    ident[:M, :M],
                )
            et_sb = etp.tile([128, T_SUB * M], bf16, tag="et", name="et")
            nc.vector.tensor_copy(et_sb[:], et_ps[:])
            st[i]["et"] = et_sb

        def stage_c2(i):
            bb, n = macros[i]
            x_bf = st[i]["x"]
            qb = st[i]["qb"]
            et_sb = st[i]["et"]
            pool_ps = batch[bb]["pl"]
            # 4-way column-tiled: each q-block runs in its own 32-col group
            # of the PE array with its own XBUS stream; the 4 streams run
            # concurrently. Partial sums land on psum partitions 32q..32q+15
            # and are reduced once per batch.
            for t in range(T_SUB):
                nc.tensor.matmul(
                    pool_ps[32 * t:32 * t + M, :],
                    lhsT=et_sb[:, t * M:(t + 1) * M],
                    rhs=x_bf[:, qb + t, :],
                    start=(n == 0),
                    stop=(n == n_mac - 1),
                    tile_position=(0, 32 * t),
                    skip_group_check=True,
                )
            del st[i]
            ci, j = macro_chunk[i]
            if j == sched[ci][2] - 1:
                del chunk_tiles[ci]
            if n == n_mac - 1:
                recip = batch[bb]["recip"]
                # fused: o = sum_q partial_q * recip, one op per partial
                o_sb = outp.tile([M, D], f32, tag="o", name="o_sb")
                nc.vector.tensor_scalar_mul(o_sb[:], pool_ps[0:M, :], recip[:])
                for q in range(1, T_SUB):
                    nc.vector.scalar_tensor_tensor(
                        o_sb[:], pool_ps[32 * q:32 * q + M, :], recip[:],
                        o_sb[:], op0=mybir.AluOpType.mult,
                        op1=mybir.AluOpType.add,
                    )
                nc.scalar.dma_start(out=out_ap[bb], in_=o_sb[:])
                del batch[bb]

        for i in range(NM + 3):
            if i < NM and i in first_of_chunk:
                c = first_of_chunk[i] + PREFETCH
                if c < len(sched):
                    stage_dma(c)
            if 3 <= i <= NM + 2:
                stage_c1(i - 3)
            if 2 <= i <= NM + 1:
                stage_b(i - 2)
            if 3 <= i <= NM + 2:
                stage_c2(i - 3)
            if i < NM:
                stage_a(i)


def build_bass(b_loc, s):
    nc = bacc.Bacc(
        "TRN2", target_bir_lowering=False, debug=False, num_devices=N_CORES
    )
    x_d = nc.dram_tensor("x", [b_loc, s, D], f32, kind="ExternalInput")
    seeds_d = nc.dram_tensor("seeds", [M, D], f32, kind="ExternalInput")
    out_d = nc.dram_tensor("out", [b_loc, M, D], f32, kind="ExternalOutput")
    with tile.TileContext(nc) as tc:
        kernel_body(tc, out_d.ap(), x_d.ap(), seeds_d.ap(), b_loc, s)
    nc.compile()
    return nc


_cached = {}


def get_nc(b_loc, s):
    key = (b_loc, s)
    if key not in _cached:
        _cached[key] = build_bass(b_loc, s)
    return _cached[key]


def kernel(x, seeds, trace=False):
    assert x.shape == (B, S, D) and seeds.shape == (M, D)
    x = np.asarray(x, dtype=np.float32)
    seeds = np.asarray(seeds, dtype=np.float32)
    b_loc = B // N_CORES
    nc = get_nc(b_loc, S)
    in_maps = [
        {
            "x": np.ascontiguousarray(x[i * b_loc:(i + 1) * b_loc]),
            "seeds": seeds,
        }
        for i in range(N_CORES)
    ]
    res = run_bass_kernel_spmd(
        nc, in_maps, core_ids=list(range(N_CORES)), trace=trace
    )
    out = np.concatenate([r["out"] for r in res.results], axis=0)
    if trace:
        kernel.last_result = res
    return out.astype(np.float32)


kernel.last_result = None


# revision 14
# speedup vs baseline: 73556.6437x; 1.1194x over previous
"""Attention-pooling kernel for TRN2 (8 NeuronCores, data-parallel over batch).

Computes, per batch b:
    scores = seeds @ x[b].T          # [M, S]
    weights = softmax(scores, -1)
    out[b] = weights @ x[b]          # [M, D]

Sharding: batch B=32 split 4-per-core across 8 cores; seeds replicated.

Per-core pipeline (all bf16 on-chip, f32 PSUM accumulation):
  - SWDGE cast-DMA loads x HBM f32 -> SBUF bf16 in tapered chunks
    (1-4 MB: small at the very start to fill the pipeline early and at
    the very end to shrink the post-stream compute tail; 4 MB in the
    middle where only line rate matters). The x stream is the critical
    path (~67 MB at ~337 GB/s effective HBM rate = ~199 us), so the
    chunk DMAs are the first instructions on the gpsimd queue and the
    seeds load rides the HWDGE (scalar) queue instead.
  - PE transposes x 128x128 blocks (bf16) -> psum -> DVE copies to SBUF
    as x^T chunks.
  - scores: 4 accumulating matmuls lhsT=seedsT chunk [128,16], rhs=xT.
  - exp on ACT straight out of PSUM, with fused accum_out row-sums
    (no max subtraction: scores = seeds.x are bounded ~|8|, exp is safe
    in f32).
  - PE transposes exp [16,128] -> expT [128,16]; pooled matmuls are
    4-way column-tiled (concurrent PE col-groups), partials accumulate
    in psum[32q:32q+16, :] over the whole batch.
  - batch end: reduce partials + recip(sum) on DVE, scale, DMA out f32.
  - Stages are software-pipelined (C(i-2), B(i-1), A(i)) so PE never
    waits on the ACT/DVE round trip of the same macro-tile.
"""

from contextlib import ExitStack

import numpy as np

import concourse.mybir as mybir
import concourse.tile as tile
from concourse import bacc
from concourse.bass_utils import run_bass_kernel_spmd
from concourse.masks import make_identity

N_CORES = 8
B, S, D, M = 32, 8192, 512, 16
S_MACRO = 512           # s rows per macro-tile
T_SUB = S_MACRO // 128  # 128-row subtiles per macro-tile
DC = D // 128            # 128-col d chunks
XP_BUFS = 12             # 1 MB chunk tiles in flight (4 KB/partition each)

f32 = mybir.dt.float32
bf16 = mybir.dt.bfloat16


def chunk_sizes(bb, b_loc, n_mac):
    """Macro counts per DMA chunk for one batch (sums to n_mac).

    Uniform 1-macro (1 MB) chunks: the SWDGE stream pipelines packets
    continuously regardless of chunk size, and per-macro completion
    granularity keeps the PE's data waits tiny (~0.3 us) so the PE_HAM
    activity monitor never sees an idle window and never half-clocks
    the PE (4 MB chunks caused ~2 us waits -> K=4 windows -> ~2.5 us
    of extra PE time after every chunk boundary).
    """
    return [1] * n_mac


def kernel_body(tc, out_ap, x_ap, seeds_ap, b_loc, s):
    nc = tc.nc
    n_mac = s // S_MACRO
    with ExitStack() as ctx:
        const = ctx.enter_context(tc.tile_pool(name="const", bufs=1))
        xp = ctx.enter_context(tc.tile_pool(name="xp", bufs=XP_BUFS))
        xtp = ctx.enter_context(tc.tile_pool(name="xtp", bufs=4))
        ep = ctx.enter_context(tc.tile_pool(name="ep", bufs=4))
        etp = ctx.enter_context(tc.tile_pool(name="etp", bufs=4))
        statp = ctx.enter_context(tc.tile_pool(name="statp", bufs=4))
        outp = ctx.enter_context(tc.tile_pool(name="outp", bufs=2))
        ps_xt = ctx.enter_context(tc.tile_pool(name="ps_xt", bufs=4, space="PSUM"))
        ps_sc = ctx.enter_context(tc.tile_pool(name="ps_sc", bufs=1, space="PSUM"))
        ps_et = ctx.enter_context(tc.tile_pool(name="ps_et", bufs=1, space="PSUM"))
        ps_pl = ctx.enter_context(tc.tile_pool(name="ps_pl", bufs=2, space="PSUM"))

        # x view per batch: [b, p, q, d] with s = p*(s/128) + q. Partition p
        # holds s/128=64 consecutive s rows (128 KB contiguous HBM), so any
        # q-slice is a contiguous per-partition read. The s-order is a fixed
        # permutation; softmax is permutation-invariant and scores/exp/pooled
        # all use the same block mapping, so it cancels.
        x_rb = x_ap.rearrange("b (p q) d -> b p q d", p=128)

        # chunk schedule: (bb, first macro in batch, n macros)
        sched = []
        for bb in range(b_loc):
            m0 = 0
            for sz in chunk_sizes(bb, b_loc, n_mac):
                sched.append((bb, m0, sz))
                m0 += sz
        first_of_chunk = {}  # global macro idx -> chunk idx
        macro_chunk = {}     # global macro idx -> (chunk idx, sub idx)
        for ci, (bb, m0, nm) in enumerate(sched):
            first_of_chunk[bb * n_mac + m0] = ci
            for j in range(nm):
                macro_chunk[bb * n_mac + m0 + j] = (ci, j)

        chunk_tiles = {}

        def stage_dma(c):
            bb, m0, nm = sched[c]
            x_bf = xp.tile([128, T_SUB, D], bf16, tag="x", name="x_bf")
            nc.gpsimd.dma_start(
                out=x_bf[:, :nm * T_SUB, :],
                in_=x_rb[bb, :, m0 * T_SUB:(m0 + nm) * T_SUB, :],
            )
            chunk_tiles[c] = x_bf

        # x chunk DMAs are the first gpsimd-queue work so SDMA starts
        # streaming immediately; identity (also gpsimd: memset +
        # affine_select) slots in after two emissions, well before the
        # first transposes need it.
        PREFETCH = XP_BUFS - 1
        stage_dma(0)
        stage_dma(1)

        ident = const.tile([128, 128], bf16)
        make_identity(nc, ident)

        # seeds on the HWDGE (scalar) queue, f32; DVE casts to bf16.
        seeds_f = const.tile([M, D], f32)
        nc.scalar.dma_start(out=seeds_f[:], in_=seeds_ap)
        seeds_bf = const.tile([M, D], bf16)
        nc.vector.tensor_copy(seeds_bf[:], seeds_f[:])

        for c in range(2, min(PREFETCH, len(sched))):
            stage_dma(c)

        # seeds -> seedsT [d, m] chunks, [128, DC*M] (dc-major)
        ps_st = ps_et.tile([128, DC * M], bf16, tag="et", name="et")
        for dc in range(DC):
            nc.tensor.transpose(
                ps_st[:, dc * M:(dc + 1) * M],
                seeds_bf[:, dc * 128:(dc + 1) * 128],
                ident[:M, :M],
            )
        seedsT = const.tile([128, DC * M], bf16)
        nc.vector.tensor_copy(seedsT[:], ps_st[:])

        # Software-pipelined across all (batch, macro) pairs:
        #   DMA: one cast-load per chunk, PREFETCH chunks ahead
        #   stage A(i): PE x-transposes, DVE psum->sbuf copies
        #   stage B(i): scores matmuls, ACT exp (+row-sum)
        #   stage C(i): PE exp-transposes, DVE copy, pooled matmuls, finalize
        # B runs 2 macros behind A and C 3 behind, so every cross-engine
        # round trip (PE->DVE xt copy->scores; ACT exp->eT) has a full
        # macro of slack and the PE never stalls mid-macro.
        macros = [(bb, n) for bb in range(b_loc) for n in range(n_mac)]
        NM = len(macros)
        st = {}  # per-macro live tiles
        batch = {}  # per-batch state: sums tile, pool psum

        def stage_a(i):
            ci, j = macro_chunk[i]
            qb = j * T_SUB
            x_bf = chunk_tiles[ci]
            xt_sb = xtp.tile([128, DC, S_MACRO], bf16, tag="xt", name="xt")
            for ph in range(DC // 2):  # 2 dc chunks per psum bank
                xt_ps = ps_xt.tile([128, 2 * S_MACRO], bf16, tag="xt", name="xt")
                for dch in range(2):
                    dc = ph * 2 + dch
                    for t in range(T_SUB):
                        nc.tensor.transpose(
                            xt_ps[:, dch * S_MACRO + t * 128:
                                  dch * S_MACRO + (t + 1) * 128],
                            x_bf[:, qb + t, dc * 128:(dc + 1) * 128],
                            ident[:],
                        )
                nc.vector.tensor_copy(xt_sb[:, ph * 2:(ph + 1) * 2, :], xt_ps[:])
            st[i] = {"x": x_bf, "xt": xt_sb, "qb": qb}

        def stage_b(i):
            bb, n = macros[i]
            if n == 0:
                batch[bb] = {"sums": statp.tile([M, n_mac], f32, tag="sums", name="sums")}
            xt_sb = st[i]["xt"]
            sc_ps = ps_sc.tile([M, S_MACRO], f32, tag="sc", name="sc")
            for dc in range(DC):
                nc.tensor.matmul(
                    sc_ps[:],
                    lhsT=seedsT[:, dc * M:(dc + 1) * M],
                    rhs=xt_sb[:, dc, :],
                    start=(dc == 0),
                    stop=(dc == DC - 1),
                )
            e_bf = ep.tile([M, S_MACRO], bf16, tag="e", name="e_bf")
            nc.scalar.activation(
                e_bf[:], sc_ps[:], mybir.ActivationFunctionType.Exp,
                accum_out=batch[bb]["sums"][:, n:n + 1],
            )
            st[i]["e"] = e_bf
            if n == n_mac - 1:
                # sums complete after this exp; compute recip early so the
                # batch-end finalize chain starts without it
                total = statp.tile([M, 1], f32, tag="tot", name="tot")
                nc.vector.reduce_sum(
                    total[:], batch[bb]["sums"][:], axis=mybir.AxisListType.X)
                recip = statp.tile([M, 1], f32, tag="rec", name="rec")
                nc.vector.reciprocal(recip[:], total[:])
                batch[bb]["recip"] = recip

        def stage_c1(i):
            # expT transposes + small DVE copy; runs while scores(i+1) stream
            bb, n = macros[i]
            if n == 0:
                batch[bb]["pl"] = ps_pl.tile([128, D], f32, tag="pl", name="pl")
            e_bf = st[i]["e"]
            et_ps = ps_et.tile([128, T_SUB * M], bf16, tag="et", name="et")
            for t in range(T_SUB):
                nc.tensor.transpose(
                    et_ps[:, t * M:(t + 1) * M],
                    e_bf[:, t * 128:(t + 1) * 128],
                    ident[:M, :M],
                )
            et_sb = etp.tile([128, T_SUB * M], bf16, tag="et", name="et")
            nc.vector.tensor_copy(et_sb[:], et_ps[:])
            st[i]["et"] = et_sb

        def stage_c2(i):
            bb, n = macros[i]
            x_bf = st[i]["x"]
            qb = st[i]["qb"]
            et_sb = st[i]["et"]
            pool_ps = batch[bb]["pl"]
            # 2-way column-tiled: the two col-groups run concurrently on
            # the PE (2 serial matmuls each), and only 2 psum partials per
            # batch remain, so the batch-end finalize is one ACT op plus
            # one DVE op instead of a 4-deep serial DVE chain (~2 us less
            # exposed tail on the last batch).
            for t in range(T_SUB):
                g = t // 2
                nc.tensor.matmul(
                    pool_ps[64 * g:64 * g + M, :],
                    lhsT=et_sb[:, t * M:(t + 1) * M],
                    rhs=x_bf[:, qb + t, :],
                    start=(n == 0 and t % 2 == 0),
                    stop=(n == n_mac - 1 and t % 2 == 1),
                    tile_position=(0, 64 * g),
                    skip_group_check=True,
                )
            del st[i]
            ci, j = macro_chunk[i]
            if j == sched[ci][2] - 1:
                del chunk_tiles[ci]
            if n == n_mac - 1:
                recip = batch[bb]["recip"]
                # o = (partial0 + partial1) * recip, split ACT/DVE
                o_sb = outp.tile([M, D], f32, tag="o", name="o_sb")
                nc.scalar.activation(
                    o_sb[:], pool_ps[0:M, :],
                    mybir.ActivationFunctionType.Copy, scale=recip[:],
                )
                nc.vector.scalar_tensor_tensor(
                    o_sb[:], pool_ps[64:64 + M, :], recip[:],
                    o_sb[:], op0=mybir.AluOpType.mult,
                    op1=mybir.AluOpType.add,
                )
                nc.scalar.dma_start(out=out_ap[bb], in_=o_sb[:])
                del batch[bb]

        for i in range(NM + 3):
            if i < NM and i in first_of_chunk:
                c = first_of_chunk[i] + PREFETCH
                if c < len(sched):
                    stage_dma(c)
            if 3 <= i <= NM + 2:
                stage_c1(i - 3)
            if 2 <= i <= NM + 1:
                stage_b(i - 2)
            if 3 <= i <= NM + 2:
                stage_c2(i - 3)
            if i < NM:
                stage_a(i)


def build_bass(b_loc, s):
    nc = bacc.Bacc(
        "TRN2", target_bir_lowering=False, debug=False, num_devices=N_CORES
    )
    x_d = nc.dram_tensor("x", [b_loc, s, D], f32, kind="ExternalInput")
    seeds_d = nc.dram_tensor("seeds", [M, D], f32, kind="ExternalInput")
    out_d = nc.dram_tensor("out", [b_loc, M, D], f32, kind="ExternalOutput")
    with tile.TileContext(nc) as tc:
        kernel_body(tc, out_d.ap(), x_d.ap(), seeds_d.ap(), b_loc, s)
    nc.compile()
    return nc


_cached = {}


def get_nc(b_loc, s):
    key = (b_loc, s)
    if key not in _cached:
        _cached[key] = build_bass(b_loc, s)
    return _cached[key]


def kernel(x, seeds, trace=False):
    assert x.shape == (B, S, D) and seeds.shape == (M, D)
    x = np.asarray(x, dtype=np.float32)
    seeds = np.asarray(seeds, dtype=np.float32)
    b_loc = B // N_CORES
    nc = get_nc(b_loc, S)
    in_maps = [
        {
            "x": np.ascontiguousarray(x[i * b_loc:(i + 1) * b_loc]),
            "seeds": seeds,
        }
        for i in range(N_CORES)
    ]
    res = run_bass_kernel_spmd(
        nc, in_maps, core_ids=list(range(N_CORES)), trace=trace
    )
    out = np.concatenate([r["out"] for r in res.results], axis=0)
    if trace:
        kernel.last_result = res
    return out.astype(np.float32)


kernel.last_result = None
